# revision 2
# baseline (speedup 1.0000x reference)
"""HGNN conv kernel for 8 Trainium2 NeuronCores — v2.

out = segment_sum(g_vals * (x @ W + b)[g_cols], g_rows, N)
reordered as out = (G @ x) @ W + rowsum(G) outer b; dest rows sharded
across 8 cores, x replicated, per-dest-tile source gather via SWDGE
dma_gather.

v4 = v2 + per-core runtime descriptor trim:
  - deeper tile pools (R bufs=4, idx bufs=8) for more DMA lookahead.
  - A one-hot build fully in bf16 (iota/dst bf16) -> 2x DVE rate.
  - S/transpose path in bf16; out written bf16, upcast on host.
  - each gather call passes num_idxs_reg loaded at runtime from a
    per-core count table, so pad slots (cross-core-max chunk padding
    plus in-chunk tail padding, ~11% of rows) generate no descriptors
    and move no bytes. Pad idx entries are -1; stale R rows are nulled
    by A's val=0 columns.
"""

import sys

import numpy as np

sys.path.insert(0, "/opt/trn_rl_repo")

import concourse.bacc as bacc
import concourse.bass as bass
import concourse.mybir as mybir
import concourse.tile as tile
from concourse.bass_utils import run_bass_kernel_spmd


def _install_ntff_hook():
    import types
    if "antenv.axon_hooks" in sys.modules:
        return
    mod = types.ModuleType("antenv.axon_hooks")
    _h = [None]
    mod.set_axon_ntff_profile_hook = lambda h: _h.__setitem__(0, h)
    mod.get_axon_ntff_profile_hook = lambda: _h[0]
    sys.modules["antenv.axon_hooks"] = mod
    import antenv
    antenv.axon_hooks = mod
    from trn_agent_boot.trn_boot import _ntff_profile_via_ctypes
    mod.set_axon_ntff_profile_hook(
        _ntff_profile_via_ctypes("/opt/axon/libaxon_pjrt.so")
    )


_install_ntff_hook()

N = 100000
F = 512
CORES = 8
RPC = 12500            # dest rows per core
TILES = 98             # ceil(12500 / 128)
NPAD = TILES * 128     # 12544
SRC_CHUNK = 25000
GROUPS = 4
GW = SRC_CHUNK + 1     # group window rows incl. one zero pad row
XROWS = GROUPS * GW    # 100004
PAD_LOCAL = SRC_CHUNK  # local index of the zero pad row in each window

F32 = mybir.dt.float32
BF16 = mybir.dt.bfloat16
I16 = mybir.dt.int16
MMDT = BF16
import ml_dtypes
NPDT = ml_dtypes.bfloat16


def _preprocess(x, g_rows, g_cols, g_vals):
    """Sort/pad edges into the per-core, per-tile, per-group chunk layout."""
    rows = np.asarray(g_rows, dtype=np.int64)
    cols = np.asarray(g_cols, dtype=np.int64)
    vals = np.asarray(g_vals, dtype=np.float32)

    core = rows // RPC
    rl = rows - core * RPC          # 0..12499 local dest row
    tile_i = rl >> 7
    grp = cols // SRC_CHUNK
    sloc = (cols - grp * SRC_CHUNK).astype(np.int16)

    key = ((core * TILES + tile_i) * GROUPS + grp) * SRC_CHUNK + (cols - grp * SRC_CHUNK)
    order = np.argsort(key, kind="stable")

    bucket = (core * TILES + tile_i) * GROUPS + grp
    cnt = np.bincount(bucket, minlength=CORES * TILES * GROUPS).reshape(
        CORES, TILES * GROUPS
    )
    n_chunks = -(-cnt.max(axis=0) // 128)            # [TILES*GROUPS]
    TC = int(n_chunks.sum())
    col_off = np.zeros(TILES * GROUPS + 1, np.int64)
    np.cumsum(n_chunks, out=col_off[1:])
    slot_off = col_off * 128
    SLOTS = TC * 128

    core_cnt = np.bincount(core, minlength=CORES)
    core_start = np.zeros(CORES + 1, np.int64)
    np.cumsum(core_cnt, out=core_start[1:])

    gidx = np.empty((CORES, 128, TC * 8), np.int16)
    gdst = np.empty((CORES, 128, TC), np.float32)
    gval = np.empty((CORES, 128, TC), np.float32)
    rsum = np.zeros((CORES, NPAD), np.float32)

    # per-core valid count per gather call, in program emission order
    calls = []  # (tg_flat_index, b0, nb)
    for t in range(TILES):
        for g in range(GROUPS):
            n = n_chunks[t * GROUPS + g]
            for b0 in range(0, n, 8):
                calls.append((t * GROUPS + g, b0, min(8, n - b0)))
    ncalls = len(calls)
    gcnt = np.zeros((CORES, 1, ncalls), np.int32)
    for c in range(CORES):
        for i, (tg_i, b0, nb) in enumerate(calls):
            v = int(cnt[c][tg_i]) - b0 * 128
            gcnt[c][0][i] = max(0, min(nb * 128, v))

    nch = n_chunks
    for c in range(CORES):
        seg = order[core_start[c]:core_start[c + 1]]
        tg = tile_i[seg] * GROUPS + grp[seg]
        cnt_tg = np.bincount(tg, minlength=TILES * GROUPS)
        gstart = np.zeros(TILES * GROUPS, np.int64)
        np.cumsum(cnt_tg[:-1], out=gstart[1:])
        pos = np.arange(len(seg), dtype=np.int64) - np.repeat(gstart, cnt_tg)
        slot = slot_off[tg] + pos

        idx_flat = np.full(SLOTS, -1, np.int16)
        idx_flat[slot] = sloc[seg]
        d_flat = np.zeros(SLOTS, np.float32)
        d_flat[slot] = (rl[seg] & 127).astype(np.float32)
        v_flat = np.zeros(SLOTS, np.float32)
        v_flat[slot] = vals[seg]

        gdst[c] = d_flat.reshape(TC, 128).T
        gval[c] = v_flat.reshape(TC, 128).T
        for tg_i in range(TILES * GROUPS):
            n = nch[tg_i]
            if n == 0:
                continue
            a = slot_off[tg_i]
            bcol = col_off[tg_i] * 8
            blk = idx_flat[a:a + n * 128].reshape(n * 8, 16).T
            gidx[c][:, bcol:bcol + n * 8] = np.tile(blk, (8, 1))

        rs = np.bincount(rl[seg], weights=vals[seg].astype(np.float64),
                         minlength=RPC)
        rsum[c][:RPC] = rs.astype(np.float32)

    return (n_chunks.reshape(TILES, GROUPS), TC, gidx, gdst, gval,
            rsum.reshape(CORES, TILES, 128), gcnt)


def _build_program(n_chunks, TC, ncalls):
    nch = n_chunks  # [TILES, GROUPS]
    GMAX = int(nch.max())
    TMAX = int(nch.sum(axis=1).max())

    nc = bacc.Bacc(
        "TRN2",
        target_bir_lowering=False,
        debug=False,
        enable_asserts=False,
        num_devices=CORES,
        num_swdge_queues=4,
    )
    xdev = nc.dram_tensor("xdev", [XROWS, F], MMDT, kind="ExternalInput").ap()
    gidx = nc.dram_tensor("gidx", [128, TC * 8], I16, kind="ExternalInput").ap()
    gdst = nc.dram_tensor("gdst", [128, TC], MMDT, kind="ExternalInput").ap()
    gval = nc.dram_tensor("gval", [128, TC], MMDT, kind="ExternalInput").ap()
    wmat = nc.dram_tensor("wmat", [F, F], MMDT, kind="ExternalInput").ap()
    bvec = nc.dram_tensor("bvec", [1, F], MMDT, kind="ExternalInput").ap()
    rsum = nc.dram_tensor("rsum", [TILES, 128], MMDT, kind="ExternalInput").ap()
    iot = nc.dram_tensor("iot", [128, 128], MMDT, kind="ExternalInput").ap()
    identt = nc.dram_tensor("identt", [128, 128], MMDT, kind="ExternalInput").ap()
    gcnt = nc.dram_tensor("gcnt", [1, ncalls], mybir.dt.int32,
                          kind="ExternalInput").ap()
    out = nc.dram_tensor("out", [NPAD, F], MMDT, kind="ExternalOutput").ap()

    from contextlib import ExitStack

    with tile.TileContext(nc) as tc, ExitStack() as ctx:
        cpool = ctx.enter_context(tc.tile_pool(name="const", bufs=1))
        idxp = ctx.enter_context(tc.tile_pool(name="idxp", bufs=8))
        dvp = ctx.enter_context(tc.tile_pool(name="dvp", bufs=3))
        rpool = ctx.enter_context(tc.tile_pool(name="rp", bufs=4))
        apool = ctx.enter_context(tc.tile_pool(name="ap", bufs=2))
        spool = ctx.enter_context(tc.tile_pool(name="sp", bufs=2))
        opool = ctx.enter_context(tc.tile_pool(name="op", bufs=2))
        psS = ctx.enter_context(tc.tile_pool(name="psS", bufs=2, space="PSUM"))
        psT = ctx.enter_context(tc.tile_pool(name="psT", bufs=2, space="PSUM"))
        psO = ctx.enter_context(tc.tile_pool(name="psO", bufs=2, space="PSUM"))

        w_t = cpool.tile([128, 4, F], MMDT)
        for k in range(4):
            nc.sync.dma_start(w_t[:, k, :], wmat[k * 128:(k + 1) * 128, :])
        b_t = cpool.tile([1, F], MMDT)
        nc.sync.dma_start(b_t[:], bvec[:])
        io_t = cpool.tile([128, 128], MMDT)
        nc.sync.dma_start(io_t[:], iot[:])
        id_t = cpool.tile([128, 128], MMDT)
        nc.sync.dma_start(id_t[:], identt[:])
        cnt_t = cpool.tile([1, ncalls], mybir.dt.int32)
        nc.sync.dma_start(cnt_t[:], gcnt[:])

        qn = 0
        call_i = 0
        r_alloc = 0
        c0 = 0
        for t in range(TILES):
            tc_t = int(nch[t].sum())
            pS = psS.tile([128, F], F32)
            rs_t = dvp.tile([1, 128], MMDT, tag="rs")
            nc.sync.dma_start(rs_t[:], rsum[t:t + 1, :])
            dst_t = dvp.tile([128, TMAX], MMDT, tag="dst")
            nc.sync.dma_start(dst_t[:, :tc_t], gdst[:, c0:c0 + tc_t])
            val_t = dvp.tile([128, TMAX], MMDT, tag="val")
            nc.sync.dma_start(val_t[:, :tc_t], gval[:, c0:c0 + tc_t])
            A = apool.tile([128, TMAX, 128], MMDT)
            nc.vector.tensor_tensor(
                out=A[:, :tc_t, :],
                in0=io_t[:].unsqueeze(1).to_broadcast([128, tc_t, 128]),
                in1=dst_t[:, :tc_t].unsqueeze(2).to_broadcast([128, tc_t, 128]),
                op=mybir.AluOpType.is_equal,
            )
            nc.vector.tensor_tensor(
                out=A[:, :tc_t, :],
                in0=A[:, :tc_t, :],
                in1=val_t[:, :tc_t].unsqueeze(2).to_broadcast([128, tc_t, 128]),
                op=mybir.AluOpType.mult,
            )
            kk = 0
            for g in range(GROUPS):
                n = int(nch[t][g])
                if n == 0:
                    continue
                it = idxp.tile([128, GMAX * 8], I16)
                nc.sync.dma_start(
                    it[:, :n * 8], gidx[:, (c0 + kk) * 8:(c0 + kk + n) * 8]
                )
                R = rpool.tile([128, GMAX, F], MMDT)
                if r_alloc < 4:
                    # first pass through the 4-buffer ring: clear initial
                    # SBUF garbage (can be NaN) in slots the trimmed
                    # gathers skip; later reuses hold finite stale rows
                    nc.vector.memset(R[:], 0.0)
                r_alloc += 1
                # ucode caps one dma_gather at 1024 indices (8 chunks)
                for b0 in range(0, n, 8):
                    nb = min(8, n - b0)
                    reg = nc.gpsimd.value_load(cnt_t[0:1, call_i:call_i + 1])
                    nc.gpsimd.dma_gather(
                        out_ap=R[:, b0:b0 + nb, :],
                        in_ap=xdev[g * GW:(g + 1) * GW, :],
                        idxs_ap=it[:, b0 * 8:(b0 + nb) * 8],
                        num_idxs=nb * 128,
                        num_idxs_reg=reg,
                        elem_size=F,
                        queue_num=qn,
                    )
                    qn = (qn + 1) % 4
                    call_i += 1
                for k in range(n):
                    nc.tensor.matmul(
                        pS[:],
                        lhsT=A[:, kk + k, :],
                        rhs=R[:, k, :],
                        start=(kk + k == 0),
                        stop=(kk + k == tc_t - 1),
                    )
                kk += n

            S = spool.tile([128, F], MMDT, tag="S")
            nc.vector.tensor_copy(S[:], pS[:])
            pT = psT.tile([128, F], MMDT)
            for k in range(4):
                nc.tensor.transpose(
                    pT[:, k * 128:(k + 1) * 128], S[:, k * 128:(k + 1) * 128], id_t[:]
                )
            ST = spool.tile([128, F], MMDT, tag="ST")
            nc.vector.tensor_copy(ST[:], pT[:])
            pO = psO.tile([128, F], F32)
            for k in range(4):
                nc.tensor.matmul(
                    pO[:],
                    lhsT=ST[:, k * 128:(k + 1) * 128],
                    rhs=w_t[:, k, :],
                    start=(k == 0),
                    stop=False,
                )
            nc.tensor.matmul(
                pO[:],
                lhsT=rs_t[0:1, :],
                rhs=b_t[0:1, :],
                start=False,
                stop=True,
            )
            O = opool.tile([128, F], MMDT)
            nc.vector.tensor_copy(O[:], pO[:])
            nc.sync.dma_start(out[t * 128:(t + 1) * 128, :], O[:])
            c0 += tc_t

    nc.compile()
    return nc


def kernel(x, g_rows, g_cols, g_vals, weight, b, trace=False):
    x = np.asarray(x, dtype=np.float32)
    weight = np.asarray(weight, dtype=np.float32)
    b = np.asarray(b, dtype=np.float32)

    (n_chunks, TC, gidx, gdst, gval, rsum, gcnt) = _preprocess(
        x, g_rows, g_cols, g_vals)

    x_dev = np.zeros((XROWS, F), NPDT)
    for g in range(GROUPS):
        x_dev[g * GW:g * GW + SRC_CHUNK] = x[g * SRC_CHUNK:(g + 1) * SRC_CHUNK]
    iota2 = np.broadcast_to(
        np.arange(128, dtype=np.float32)[None, :], (128, 128)
    ).astype(NPDT).copy()
    ident = np.eye(128, dtype=np.float32).astype(NPDT)

    nc = _build_program(n_chunks, TC, gcnt.shape[2])

    in_maps = []
    for c in range(CORES):
        in_maps.append({
            "xdev": x_dev,
            "gidx": gidx[c],
            "gdst": gdst[c].astype(NPDT),
            "gval": gval[c].astype(NPDT),
            "wmat": weight.astype(NPDT),
            "bvec": b.reshape(1, F).astype(NPDT),
            "rsum": rsum[c].astype(NPDT),
            "iot": iota2,
            "identt": ident,
            "gcnt": gcnt[c],
        })

    res = run_bass_kernel_spmd(nc, in_maps, core_ids=list(range(CORES)), trace=trace)
    outs = [res.results[c]["out"][:RPC].astype(np.float32) for c in range(CORES)]
    full = np.concatenate(outs, axis=0)
    kernel.last_exec_time_ns = res.exec_time_ns
    kernel.last_results = res
    return full


# revision 3
# speedup vs baseline: 1.0827x; 1.0827x over previous
"""HGNN conv kernel for 8 Trainium2 NeuronCores — v2.

out = segment_sum(g_vals * (x @ W + b)[g_cols], g_rows, N)
reordered as out = (G @ x) @ W + rowsum(G) outer b; dest rows sharded
across 8 cores, x replicated, per-dest-tile source gather via SWDGE
dma_gather.

v6 = v4 + greedy per-queue load balancing (big calls were
parity-locked onto queues 1/3, 6x the bytes of queues 0/2):
  - deeper tile pools (R bufs=4, idx bufs=8) for more DMA lookahead.
  - A one-hot build fully in bf16 (iota/dst bf16) -> 2x DVE rate.
  - S/transpose path in bf16; out written bf16, upcast on host.
  - each gather call passes num_idxs_reg loaded at runtime from a
    per-core count table, so pad slots (cross-core-max chunk padding
    plus in-chunk tail padding, ~11% of rows) generate no descriptors
    and move no bytes. Pad idx entries are -1; stale R rows are nulled
    by A's val=0 columns.
"""

import sys

import numpy as np

sys.path.insert(0, "/opt/trn_rl_repo")

import concourse.bacc as bacc
import concourse.bass as bass
import concourse.mybir as mybir
import concourse.tile as tile
from concourse.bass_utils import run_bass_kernel_spmd


def _install_ntff_hook():
    import types
    if "antenv.axon_hooks" in sys.modules:
        return
    mod = types.ModuleType("antenv.axon_hooks")
    _h = [None]
    mod.set_axon_ntff_profile_hook = lambda h: _h.__setitem__(0, h)
    mod.get_axon_ntff_profile_hook = lambda: _h[0]
    sys.modules["antenv.axon_hooks"] = mod
    import antenv
    antenv.axon_hooks = mod
    from trn_agent_boot.trn_boot import _ntff_profile_via_ctypes
    mod.set_axon_ntff_profile_hook(
        _ntff_profile_via_ctypes("/opt/axon/libaxon_pjrt.so")
    )


_install_ntff_hook()

N = 100000
F = 512
CORES = 8
RPC = 12500            # dest rows per core
TILES = 98             # ceil(12500 / 128)
NPAD = TILES * 128     # 12544
SRC_CHUNK = 25000
GROUPS = 4
GW = SRC_CHUNK + 1     # group window rows incl. one zero pad row
XROWS = GROUPS * GW    # 100004
PAD_LOCAL = SRC_CHUNK  # local index of the zero pad row in each window

F32 = mybir.dt.float32
BF16 = mybir.dt.bfloat16
I16 = mybir.dt.int16
MMDT = BF16
import ml_dtypes
NPDT = ml_dtypes.bfloat16


def _preprocess(x, g_rows, g_cols, g_vals):
    """Sort/pad edges into the per-core, per-tile, per-group chunk layout."""
    rows = np.asarray(g_rows, dtype=np.int64)
    cols = np.asarray(g_cols, dtype=np.int64)
    vals = np.asarray(g_vals, dtype=np.float32)

    core = rows // RPC
    rl = rows - core * RPC          # 0..12499 local dest row
    tile_i = rl >> 7
    grp = cols // SRC_CHUNK
    sloc = (cols - grp * SRC_CHUNK).astype(np.int16)

    key = ((core * TILES + tile_i) * GROUPS + grp) * SRC_CHUNK + (cols - grp * SRC_CHUNK)
    order = np.argsort(key, kind="stable")

    bucket = (core * TILES + tile_i) * GROUPS + grp
    cnt = np.bincount(bucket, minlength=CORES * TILES * GROUPS).reshape(
        CORES, TILES * GROUPS
    )
    n_chunks = -(-cnt.max(axis=0) // 128)            # [TILES*GROUPS]
    TC = int(n_chunks.sum())
    col_off = np.zeros(TILES * GROUPS + 1, np.int64)
    np.cumsum(n_chunks, out=col_off[1:])
    slot_off = col_off * 128
    SLOTS = TC * 128

    core_cnt = np.bincount(core, minlength=CORES)
    core_start = np.zeros(CORES + 1, np.int64)
    np.cumsum(core_cnt, out=core_start[1:])

    gidx = np.empty((CORES, 128, TC * 8), np.int16)
    gdst = np.empty((CORES, 128, TC), np.float32)
    gval = np.empty((CORES, 128, TC), np.float32)
    rsum = np.zeros((CORES, NPAD), np.float32)

    # per-core valid count per gather call, in program emission order
    calls = []  # (tg_flat_index, b0, nb)
    for t in range(TILES):
        for g in range(GROUPS):
            n = n_chunks[t * GROUPS + g]
            for b0 in range(0, n, 8):
                calls.append((t * GROUPS + g, b0, min(8, n - b0)))
    ncalls = len(calls)
    gcnt = np.zeros((CORES, 1, ncalls), np.int32)
    for c in range(CORES):
        for i, (tg_i, b0, nb) in enumerate(calls):
            v = int(cnt[c][tg_i]) - b0 * 128
            gcnt[c][0][i] = max(0, min(nb * 128, v))

    nch = n_chunks
    for c in range(CORES):
        seg = order[core_start[c]:core_start[c + 1]]
        tg = tile_i[seg] * GROUPS + grp[seg]
        cnt_tg = np.bincount(tg, minlength=TILES * GROUPS)
        gstart = np.zeros(TILES * GROUPS, np.int64)
        np.cumsum(cnt_tg[:-1], out=gstart[1:])
        pos = np.arange(len(seg), dtype=np.int64) - np.repeat(gstart, cnt_tg)
        slot = slot_off[tg] + pos

        idx_flat = np.full(SLOTS, -1, np.int16)
        idx_flat[slot] = sloc[seg]
        d_flat = np.zeros(SLOTS, np.float32)
        d_flat[slot] = (rl[seg] & 127).astype(np.float32)
        v_flat = np.zeros(SLOTS, np.float32)
        v_flat[slot] = vals[seg]

        gdst[c] = d_flat.reshape(TC, 128).T
        gval[c] = v_flat.reshape(TC, 128).T
        for tg_i in range(TILES * GROUPS):
            n = nch[tg_i]
            if n == 0:
                continue
            a = slot_off[tg_i]
            bcol = col_off[tg_i] * 8
            blk = idx_flat[a:a + n * 128].reshape(n * 8, 16).T
            gidx[c][:, bcol:bcol + n * 8] = np.tile(blk, (8, 1))

        rs = np.bincount(rl[seg], weights=vals[seg].astype(np.float64),
                         minlength=RPC)
        rsum[c][:RPC] = rs.astype(np.float32)

    return (n_chunks.reshape(TILES, GROUPS), TC, gidx, gdst, gval,
            rsum.reshape(CORES, TILES, 128), gcnt)


def _build_program(n_chunks, TC, ncalls, call_load):
    nch = n_chunks  # [TILES, GROUPS]
    GMAX = int(nch.max())
    TMAX = int(nch.sum(axis=1).max())

    nc = bacc.Bacc(
        "TRN2",
        target_bir_lowering=False,
        debug=False,
        enable_asserts=False,
        num_devices=CORES,
        num_swdge_queues=4,
    )
    xdev = nc.dram_tensor("xdev", [XROWS, F], MMDT, kind="ExternalInput").ap()
    gidx = nc.dram_tensor("gidx", [128, TC * 8], I16, kind="ExternalInput").ap()
    gdst = nc.dram_tensor("gdst", [128, TC], MMDT, kind="ExternalInput").ap()
    gval = nc.dram_tensor("gval", [128, TC], MMDT, kind="ExternalInput").ap()
    wmat = nc.dram_tensor("wmat", [F, F], MMDT, kind="ExternalInput").ap()
    bvec = nc.dram_tensor("bvec", [1, F], MMDT, kind="ExternalInput").ap()
    rsum = nc.dram_tensor("rsum", [TILES, 128], MMDT, kind="ExternalInput").ap()
    iot = nc.dram_tensor("iot", [128, 128], MMDT, kind="ExternalInput").ap()
    identt = nc.dram_tensor("identt", [128, 128], MMDT, kind="ExternalInput").ap()
    gcnt = nc.dram_tensor("gcnt", [1, ncalls], mybir.dt.int32,
                          kind="ExternalInput").ap()
    out = nc.dram_tensor("out", [NPAD, F], MMDT, kind="ExternalOutput").ap()

    from contextlib import ExitStack

    with tile.TileContext(nc) as tc, ExitStack() as ctx:
        cpool = ctx.enter_context(tc.tile_pool(name="const", bufs=1))
        idxp = ctx.enter_context(tc.tile_pool(name="idxp", bufs=8))
        dvp = ctx.enter_context(tc.tile_pool(name="dvp", bufs=3))
        rpool = ctx.enter_context(tc.tile_pool(name="rp", bufs=4))
        apool = ctx.enter_context(tc.tile_pool(name="ap", bufs=2))
        spool = ctx.enter_context(tc.tile_pool(name="sp", bufs=2))
        opool = ctx.enter_context(tc.tile_pool(name="op", bufs=2))
        psS = ctx.enter_context(tc.tile_pool(name="psS", bufs=2, space="PSUM"))
        psT = ctx.enter_context(tc.tile_pool(name="psT", bufs=2, space="PSUM"))
        psO = ctx.enter_context(tc.tile_pool(name="psO", bufs=2, space="PSUM"))

        w_t = cpool.tile([128, 4, F], MMDT)
        for k in range(4):
            nc.sync.dma_start(w_t[:, k, :], wmat[k * 128:(k + 1) * 128, :])
        b_t = cpool.tile([1, F], MMDT)
        nc.sync.dma_start(b_t[:], bvec[:])
        io_t = cpool.tile([128, 128], MMDT)
        nc.sync.dma_start(io_t[:], iot[:])
        id_t = cpool.tile([128, 128], MMDT)
        nc.sync.dma_start(id_t[:], identt[:])
        cnt_t = cpool.tile([1, ncalls], mybir.dt.int32)
        nc.sync.dma_start(cnt_t[:], gcnt[:])

        qload = [0.0, 0.0, 0.0, 0.0]
        call_i = 0
        r_alloc = 0
        c0 = 0
        for t in range(TILES):
            tc_t = int(nch[t].sum())
            pS = psS.tile([128, F], F32)
            rs_t = dvp.tile([1, 128], MMDT, tag="rs")
            nc.sync.dma_start(rs_t[:], rsum[t:t + 1, :])
            dst_t = dvp.tile([128, TMAX], MMDT, tag="dst")
            nc.sync.dma_start(dst_t[:, :tc_t], gdst[:, c0:c0 + tc_t])
            val_t = dvp.tile([128, TMAX], MMDT, tag="val")
            nc.sync.dma_start(val_t[:, :tc_t], gval[:, c0:c0 + tc_t])
            A = apool.tile([128, TMAX, 128], MMDT)
            nc.vector.tensor_tensor(
                out=A[:, :tc_t, :],
                in0=io_t[:].unsqueeze(1).to_broadcast([128, tc_t, 128]),
                in1=dst_t[:, :tc_t].unsqueeze(2).to_broadcast([128, tc_t, 128]),
                op=mybir.AluOpType.is_equal,
            )
            nc.vector.tensor_tensor(
                out=A[:, :tc_t, :],
                in0=A[:, :tc_t, :],
                in1=val_t[:, :tc_t].unsqueeze(2).to_broadcast([128, tc_t, 128]),
                op=mybir.AluOpType.mult,
            )
            kk = 0
            for g in range(GROUPS):
                n = int(nch[t][g])
                if n == 0:
                    continue
                it = idxp.tile([128, GMAX * 8], I16)
                nc.sync.dma_start(
                    it[:, :n * 8], gidx[:, (c0 + kk) * 8:(c0 + kk + n) * 8]
                )
                R = rpool.tile([128, GMAX, F], MMDT)
                if r_alloc < 4:
                    # first pass through the 4-buffer ring: clear initial
                    # SBUF garbage (can be NaN) in slots the trimmed
                    # gathers skip; later reuses hold finite stale rows
                    nc.vector.memset(R[:], 0.0)
                r_alloc += 1
                # ucode caps one dma_gather at 1024 indices (8 chunks)
                for b0 in range(0, n, 8):
                    nb = min(8, n - b0)
                    reg = nc.gpsimd.value_load(cnt_t[0:1, call_i:call_i + 1])
                    qn = min(range(4), key=lambda q: qload[q])
                    qload[qn] += call_load[call_i]
                    nc.gpsimd.dma_gather(
                        out_ap=R[:, b0:b0 + nb, :],
                        in_ap=xdev[g * GW:(g + 1) * GW, :],
                        idxs_ap=it[:, b0 * 8:(b0 + nb) * 8],
                        num_idxs=nb * 128,
                        num_idxs_reg=reg,
                        elem_size=F,
                        queue_num=qn,
                    )
                    call_i += 1
                for k in range(n):
                    nc.tensor.matmul(
                        pS[:],
                        lhsT=A[:, kk + k, :],
                        rhs=R[:, k, :],
                        start=(kk + k == 0),
                        stop=(kk + k == tc_t - 1),
                    )
                kk += n

            S = spool.tile([128, F], MMDT, tag="S")
            nc.vector.tensor_copy(S[:], pS[:])
            pT = psT.tile([128, F], MMDT)
            for k in range(4):
                nc.tensor.transpose(
                    pT[:, k * 128:(k + 1) * 128], S[:, k * 128:(k + 1) * 128], id_t[:]
                )
            ST = spool.tile([128, F], MMDT, tag="ST")
            nc.vector.tensor_copy(ST[:], pT[:])
            pO = psO.tile([128, F], F32)
            for k in range(4):
                nc.tensor.matmul(
                    pO[:],
                    lhsT=ST[:, k * 128:(k + 1) * 128],
                    rhs=w_t[:, k, :],
                    start=(k == 0),
                    stop=False,
                )
            nc.tensor.matmul(
                pO[:],
                lhsT=rs_t[0:1, :],
                rhs=b_t[0:1, :],
                start=False,
                stop=True,
            )
            O = opool.tile([128, F], MMDT)
            nc.vector.tensor_copy(O[:], pO[:])
            nc.sync.dma_start(out[t * 128:(t + 1) * 128, :], O[:])
            c0 += tc_t

    nc.compile()
    return nc


def kernel(x, g_rows, g_cols, g_vals, weight, b, trace=False):
    x = np.asarray(x, dtype=np.float32)
    weight = np.asarray(weight, dtype=np.float32)
    b = np.asarray(b, dtype=np.float32)

    (n_chunks, TC, gidx, gdst, gval, rsum, gcnt) = _preprocess(
        x, g_rows, g_cols, g_vals)

    x_dev = np.zeros((XROWS, F), NPDT)
    for g in range(GROUPS):
        x_dev[g * GW:g * GW + SRC_CHUNK] = x[g * SRC_CHUNK:(g + 1) * SRC_CHUNK]
    iota2 = np.broadcast_to(
        np.arange(128, dtype=np.float32)[None, :], (128, 128)
    ).astype(NPDT).copy()
    ident = np.eye(128, dtype=np.float32).astype(NPDT)

    call_load = gcnt.mean(axis=0)[0]  # mean valid idxs per call
    nc = _build_program(n_chunks, TC, gcnt.shape[2], call_load)

    in_maps = []
    for c in range(CORES):
        in_maps.append({
            "xdev": x_dev,
            "gidx": gidx[c],
            "gdst": gdst[c].astype(NPDT),
            "gval": gval[c].astype(NPDT),
            "wmat": weight.astype(NPDT),
            "bvec": b.reshape(1, F).astype(NPDT),
            "rsum": rsum[c].astype(NPDT),
            "iot": iota2,
            "identt": ident,
            "gcnt": gcnt[c],
        })

    res = run_bass_kernel_spmd(nc, in_maps, core_ids=list(range(CORES)), trace=trace)
    outs = [res.results[c]["out"][:RPC].astype(np.float32) for c in range(CORES)]
    full = np.concatenate(outs, axis=0)
    kernel.last_exec_time_ns = res.exec_time_ns
    kernel.last_results = res
    return full


# revision 4
# speedup vs baseline: 1.0848x; 1.0020x over previous
"""HGNN conv kernel for 8 Trainium2 NeuronCores — v2.

out = segment_sum(g_vals * (x @ W + b)[g_cols], g_rows, N)
reordered as out = (G @ x) @ W + rowsum(G) outer b; dest rows sharded
across 8 cores, x replicated, per-dest-tile source gather via SWDGE
dma_gather.

v6 = v4 + greedy per-queue load balancing (big calls were
parity-locked onto queues 1/3, 6x the bytes of queues 0/2):
  - deeper tile pools (R bufs=4, idx bufs=8) for more DMA lookahead.
  - A one-hot build fully in bf16 (iota/dst bf16) -> 2x DVE rate.
  - S/transpose path in bf16; out written bf16, upcast on host.
  - each gather call passes num_idxs_reg loaded at runtime from a
    per-core count table, so pad slots (cross-core-max chunk padding
    plus in-chunk tail padding, ~11% of rows) generate no descriptors
    and move no bytes. Pad idx entries are -1; stale R rows are nulled
    by A's val=0 columns.
"""

import sys

import numpy as np

sys.path.insert(0, "/opt/trn_rl_repo")

import concourse.bacc as bacc
import concourse.bass as bass
import concourse.mybir as mybir
import concourse.tile as tile
from concourse.bass_utils import run_bass_kernel_spmd


def _install_ntff_hook():
    import types
    if "antenv.axon_hooks" in sys.modules:
        return
    mod = types.ModuleType("antenv.axon_hooks")
    _h = [None]
    mod.set_axon_ntff_profile_hook = lambda h: _h.__setitem__(0, h)
    mod.get_axon_ntff_profile_hook = lambda: _h[0]
    sys.modules["antenv.axon_hooks"] = mod
    import antenv
    antenv.axon_hooks = mod
    from trn_agent_boot.trn_boot import _ntff_profile_via_ctypes
    mod.set_axon_ntff_profile_hook(
        _ntff_profile_via_ctypes("/opt/axon/libaxon_pjrt.so")
    )


_install_ntff_hook()

N = 100000
F = 512
CORES = 8
RPC = 12500            # dest rows per core
TILES = 98             # ceil(12500 / 128)
NPAD = TILES * 128     # 12544
SRC_CHUNK = 25000
GROUPS = 4
GW = SRC_CHUNK + 1     # group window rows incl. one zero pad row
XROWS = GROUPS * GW    # 100004
PAD_LOCAL = SRC_CHUNK  # local index of the zero pad row in each window

F32 = mybir.dt.float32
BF16 = mybir.dt.bfloat16
I16 = mybir.dt.int16
MMDT = BF16
import ml_dtypes
NPDT = ml_dtypes.bfloat16


def _preprocess(x, g_rows, g_cols, g_vals):
    """Sort/pad edges into the per-core, per-tile, per-group chunk layout."""
    rows = np.asarray(g_rows, dtype=np.int64)
    cols = np.asarray(g_cols, dtype=np.int64)
    vals = np.asarray(g_vals, dtype=np.float32)

    core = rows // RPC
    rl = rows - core * RPC          # 0..12499 local dest row
    tile_i = rl >> 7
    grp = cols // SRC_CHUNK
    sloc = (cols - grp * SRC_CHUNK).astype(np.int16)

    key = ((core * TILES + tile_i) * GROUPS + grp) * SRC_CHUNK + (cols - grp * SRC_CHUNK)
    order = np.argsort(key, kind="stable")

    bucket = (core * TILES + tile_i) * GROUPS + grp
    cnt = np.bincount(bucket, minlength=CORES * TILES * GROUPS).reshape(
        CORES, TILES * GROUPS
    )
    n_chunks = -(-cnt.max(axis=0) // 128)            # [TILES*GROUPS]
    TC = int(n_chunks.sum())
    col_off = np.zeros(TILES * GROUPS + 1, np.int64)
    np.cumsum(n_chunks, out=col_off[1:])
    slot_off = col_off * 128
    SLOTS = TC * 128

    core_cnt = np.bincount(core, minlength=CORES)
    core_start = np.zeros(CORES + 1, np.int64)
    np.cumsum(core_cnt, out=core_start[1:])

    gidx = np.empty((CORES, 128, TC * 8), np.int16)
    gdst = np.empty((CORES, 128, TC), np.float32)
    gval = np.empty((CORES, 128, TC), np.float32)
    rsum = np.zeros((CORES, NPAD), np.float32)

    # per-core valid count per gather call, in program emission order
    calls = []  # (tg_flat_index, b0, nb)
    for t in range(TILES):
        for g in range(GROUPS):
            n = n_chunks[t * GROUPS + g]
            for b0 in range(0, n, 8):
                calls.append((t * GROUPS + g, b0, min(8, n - b0)))
    ncalls = len(calls)
    gcnt = np.zeros((CORES, 1, ncalls), np.int32)
    for c in range(CORES):
        for i, (tg_i, b0, nb) in enumerate(calls):
            v = int(cnt[c][tg_i]) - b0 * 128
            gcnt[c][0][i] = max(0, min(nb * 128, v))

    nch = n_chunks
    for c in range(CORES):
        seg = order[core_start[c]:core_start[c + 1]]
        tg = tile_i[seg] * GROUPS + grp[seg]
        cnt_tg = np.bincount(tg, minlength=TILES * GROUPS)
        gstart = np.zeros(TILES * GROUPS, np.int64)
        np.cumsum(cnt_tg[:-1], out=gstart[1:])
        pos = np.arange(len(seg), dtype=np.int64) - np.repeat(gstart, cnt_tg)
        slot = slot_off[tg] + pos

        idx_flat = np.full(SLOTS, -1, np.int16)
        idx_flat[slot] = sloc[seg]
        d_flat = np.zeros(SLOTS, np.float32)
        d_flat[slot] = (rl[seg] & 127).astype(np.float32)
        v_flat = np.zeros(SLOTS, np.float32)
        v_flat[slot] = vals[seg]

        gdst[c] = d_flat.reshape(TC, 128).T
        gval[c] = v_flat.reshape(TC, 128).T
        for tg_i in range(TILES * GROUPS):
            n = nch[tg_i]
            if n == 0:
                continue
            a = slot_off[tg_i]
            bcol = col_off[tg_i] * 8
            blk = idx_flat[a:a + n * 128].reshape(n * 8, 16).T
            gidx[c][:, bcol:bcol + n * 8] = np.tile(blk, (8, 1))

        rs = np.bincount(rl[seg], weights=vals[seg].astype(np.float64),
                         minlength=RPC)
        rsum[c][:RPC] = rs.astype(np.float32)

    return (n_chunks.reshape(TILES, GROUPS), TC, gidx, gdst, gval,
            rsum.reshape(CORES, TILES, 128), gcnt)


def _build_program(n_chunks, TC, ncalls, call_load):
    nch = n_chunks  # [TILES, GROUPS]
    GMAX = int(nch.max())
    TMAX = int(nch.sum(axis=1).max())

    nc = bacc.Bacc(
        "TRN2",
        target_bir_lowering=False,
        debug=False,
        enable_asserts=False,
        num_devices=CORES,
        num_swdge_queues=4,
    )
    xdev = nc.dram_tensor("xdev", [XROWS, F], MMDT, kind="ExternalInput").ap()
    gidx = nc.dram_tensor("gidx", [128, TC * 8], I16, kind="ExternalInput").ap()
    gdst = nc.dram_tensor("gdst", [128, TC], MMDT, kind="ExternalInput").ap()
    gval = nc.dram_tensor("gval", [128, TC], MMDT, kind="ExternalInput").ap()
    wmat = nc.dram_tensor("wmat", [F, F], MMDT, kind="ExternalInput").ap()
    bvec = nc.dram_tensor("bvec", [1, F], MMDT, kind="ExternalInput").ap()
    rsum = nc.dram_tensor("rsum", [TILES, 128], MMDT, kind="ExternalInput").ap()
    iot = nc.dram_tensor("iot", [128, 128], MMDT, kind="ExternalInput").ap()
    identt = nc.dram_tensor("identt", [128, 128], MMDT, kind="ExternalInput").ap()
    gcnt = nc.dram_tensor("gcnt", [1, ncalls], mybir.dt.int32,
                          kind="ExternalInput").ap()
    out = nc.dram_tensor("out", [NPAD, F], MMDT, kind="ExternalOutput").ap()

    from contextlib import ExitStack

    with tile.TileContext(nc) as tc, ExitStack() as ctx:
        cpool = ctx.enter_context(tc.tile_pool(name="const", bufs=1))
        idxp = ctx.enter_context(tc.tile_pool(name="idxp", bufs=10))
        dvp = ctx.enter_context(tc.tile_pool(name="dvp", bufs=4))
        rpool = ctx.enter_context(tc.tile_pool(name="rp", bufs=6))
        apool = ctx.enter_context(tc.tile_pool(name="ap", bufs=3))
        spool = ctx.enter_context(tc.tile_pool(name="sp", bufs=2))
        opool = ctx.enter_context(tc.tile_pool(name="op", bufs=2))
        psS = ctx.enter_context(tc.tile_pool(name="psS", bufs=2, space="PSUM"))
        psT = ctx.enter_context(tc.tile_pool(name="psT", bufs=2, space="PSUM"))
        psO = ctx.enter_context(tc.tile_pool(name="psO", bufs=2, space="PSUM"))

        w_t = cpool.tile([128, 4, F], MMDT)
        for k in range(4):
            nc.sync.dma_start(w_t[:, k, :], wmat[k * 128:(k + 1) * 128, :])
        b_t = cpool.tile([1, F], MMDT)
        nc.sync.dma_start(b_t[:], bvec[:])
        io_t = cpool.tile([128, 128], MMDT)
        nc.sync.dma_start(io_t[:], iot[:])
        id_t = cpool.tile([128, 128], MMDT)
        nc.sync.dma_start(id_t[:], identt[:])
        cnt_t = cpool.tile([1, ncalls], mybir.dt.int32)
        nc.sync.dma_start(cnt_t[:], gcnt[:])

        qload = [0.0, 0.0, 0.0, 0.0]
        call_i = 0
        r_alloc = 0
        c0 = 0
        for t in range(TILES):
            tc_t = int(nch[t].sum())
            pS = psS.tile([128, F], F32)
            rs_t = dvp.tile([1, 128], MMDT, tag="rs")
            nc.sync.dma_start(rs_t[:], rsum[t:t + 1, :])
            dst_t = dvp.tile([128, TMAX], MMDT, tag="dst")
            nc.sync.dma_start(dst_t[:, :tc_t], gdst[:, c0:c0 + tc_t])
            val_t = dvp.tile([128, TMAX], MMDT, tag="val")
            nc.sync.dma_start(val_t[:, :tc_t], gval[:, c0:c0 + tc_t])
            A = apool.tile([128, TMAX, 128], MMDT)
            nc.vector.tensor_tensor(
                out=A[:, :tc_t, :],
                in0=io_t[:].unsqueeze(1).to_broadcast([128, tc_t, 128]),
                in1=dst_t[:, :tc_t].unsqueeze(2).to_broadcast([128, tc_t, 128]),
                op=mybir.AluOpType.is_equal,
            )
            nc.vector.tensor_tensor(
                out=A[:, :tc_t, :],
                in0=A[:, :tc_t, :],
                in1=val_t[:, :tc_t].unsqueeze(2).to_broadcast([128, tc_t, 128]),
                op=mybir.AluOpType.mult,
            )
            kk = 0
            for g in range(GROUPS):
                n = int(nch[t][g])
                if n == 0:
                    continue
                it = idxp.tile([128, GMAX * 8], I16)
                nc.sync.dma_start(
                    it[:, :n * 8], gidx[:, (c0 + kk) * 8:(c0 + kk + n) * 8]
                )
                R = rpool.tile([128, GMAX, F], MMDT)
                if r_alloc < 6:
                    # first pass through the 4-buffer ring: clear initial
                    # SBUF garbage (can be NaN) in slots the trimmed
                    # gathers skip; later reuses hold finite stale rows
                    nc.vector.memset(R[:], 0.0)
                r_alloc += 1
                # ucode caps one dma_gather at 1024 indices (8 chunks)
                for b0 in range(0, n, 8):
                    nb = min(8, n - b0)
                    reg = nc.gpsimd.value_load(cnt_t[0:1, call_i:call_i + 1])
                    qn = min(range(4), key=lambda q: qload[q])
                    qload[qn] += call_load[call_i]
                    nc.gpsimd.dma_gather(
                        out_ap=R[:, b0:b0 + nb, :],
                        in_ap=xdev[g * GW:(g + 1) * GW, :],
                        idxs_ap=it[:, b0 * 8:(b0 + nb) * 8],
                        num_idxs=nb * 128,
                        num_idxs_reg=reg,
                        elem_size=F,
                        queue_num=qn,
                    )
                    call_i += 1
                for k in range(n):
                    nc.tensor.matmul(
                        pS[:],
                        lhsT=A[:, kk + k, :],
                        rhs=R[:, k, :],
                        start=(kk + k == 0),
                        stop=(kk + k == tc_t - 1),
                    )
                kk += n

            S = spool.tile([128, F], MMDT, tag="S")
            nc.vector.tensor_copy(S[:], pS[:])
            pT = psT.tile([128, F], MMDT)
            for k in range(4):
                nc.tensor.transpose(
                    pT[:, k * 128:(k + 1) * 128], S[:, k * 128:(k + 1) * 128], id_t[:]
                )
            ST = spool.tile([128, F], MMDT, tag="ST")
            nc.vector.tensor_copy(ST[:], pT[:])
            pO = psO.tile([128, F], F32)
            for k in range(4):
                nc.tensor.matmul(
                    pO[:],
                    lhsT=ST[:, k * 128:(k + 1) * 128],
                    rhs=w_t[:, k, :],
                    start=(k == 0),
                    stop=False,
                )
            nc.tensor.matmul(
                pO[:],
                lhsT=rs_t[0:1, :],
                rhs=b_t[0:1, :],
                start=False,
                stop=True,
            )
            O = opool.tile([128, F], MMDT)
            nc.vector.tensor_copy(O[:], pO[:])
            nc.sync.dma_start(out[t * 128:(t + 1) * 128, :], O[:])
            c0 += tc_t

    nc.compile()
    return nc


def kernel(x, g_rows, g_cols, g_vals, weight, b, trace=False):
    x = np.asarray(x, dtype=np.float32)
    weight = np.asarray(weight, dtype=np.float32)
    b = np.asarray(b, dtype=np.float32)

    (n_chunks, TC, gidx, gdst, gval, rsum, gcnt) = _preprocess(
        x, g_rows, g_cols, g_vals)

    x_dev = np.zeros((XROWS, F), NPDT)
    for g in range(GROUPS):
        x_dev[g * GW:g * GW + SRC_CHUNK] = x[g * SRC_CHUNK:(g + 1) * SRC_CHUNK]
    iota2 = np.broadcast_to(
        np.arange(128, dtype=np.float32)[None, :], (128, 128)
    ).astype(NPDT).copy()
    ident = np.eye(128, dtype=np.float32).astype(NPDT)

    call_load = gcnt.mean(axis=0)[0]  # mean valid idxs per call
    nc = _build_program(n_chunks, TC, gcnt.shape[2], call_load)

    in_maps = []
    for c in range(CORES):
        in_maps.append({
            "xdev": x_dev,
            "gidx": gidx[c],
            "gdst": gdst[c].astype(NPDT),
            "gval": gval[c].astype(NPDT),
            "wmat": weight.astype(NPDT),
            "bvec": b.reshape(1, F).astype(NPDT),
            "rsum": rsum[c].astype(NPDT),
            "iot": iota2,
            "identt": ident,
            "gcnt": gcnt[c],
        })

    res = run_bass_kernel_spmd(nc, in_maps, core_ids=list(range(CORES)), trace=trace)
    outs = [res.results[c]["out"][:RPC].astype(np.float32) for c in range(CORES)]
    full = np.concatenate(outs, axis=0)
    kernel.last_exec_time_ns = res.exec_time_ns
    kernel.last_results = res
    return full
